# revision 22
# baseline (speedup 1.0000x reference)
"""GCN message-passing kernel for trn2 (8 NeuronCores, SPMD + split AllGather).

v7 strategy (banded strips + 3-way AG + hop-2 dest filtering):
  - Shard the N=100352 (padded) node dim across 8 cores (12544 rows each).
  - Unified 4-window source layout: x0 is pre-permuted on host into
    [A | B1 | B2] segments matching the three AllGather outputs, so hop-1
    and hop-2 use the same (win, loc) addressing.
  - Per sg (1024 dest rows), segment-sum runs as PE matmuls into a
    [128, 1024] fp32 PSUM accumulator: per 128-edge block a narrow banded
    one-hot routing strip (width 32/64/128 covering the block's dest-row
    span) is built on DVE; pieces split at PSUM 512-col bank boundaries.
  - SWDGE descriptor generation on the Pool engine is the bottleneck
    (~2.2ns/edge, serial), so: hop-2 keeps only edges whose dest row is
    referenced by a pair index (-13.5%), the AllGather is split 3 ways
    (sgs 0-5 / 6-9 / 10-12) triggered at h1 sg7 / sg11 / end so hop-2
    windows 0-2 are gatherable immediately after hop-1, and hop-2
    prefetches two sgs of win-0/1/2 gathers before pair0 fills the
    AGb2 window.
  - x_new = (A x) @ W + b per tile by swapping matmul operand roles; norm
    squares on the Scalar engine; everything fp16 2x-mode where possible.
  - Pair streams assembled on host from per-hop normalized tables (fp16).
"""
import os
import sys

sys.path.insert(0, "/opt/trn_rl_repo")

import numpy as np

N = 100000
D = 128
NCORES = 8
SHARD = 12544            # 98 tiles of 128
NTILE = SHARD // 128     # 98
NPAD = SHARD * NCORES    # 100352
WIN = 32768
NWIN = 4
SG_TILES = 8
NSG = (NTILE + SG_TILES - 1) // SG_TILES  # 13
SGR = SG_TILES * 128     # 1024 rows per sg
HA = 6144                # sgs 0-5  -> segment A
HB = SHARD - HA          # sgs 6-12 -> segment B (6400 rows)
NA = HA * NCORES         # 49152
NB = HB * NCORES         # 51200
E_PAIR = 50000
P = 128

_CACHE = {}
LAST_RESULTS = None  # BassKernelResults of the most recent run (for test.py)


def _ceil(a, b):
    return -(-a // b)


def _pack_idx(idx_arr, cap):
    """Pack idx list (len<=cap*128, int) to the [128, cap*8] wrapped+replicated
    int16 layout. Pads with 0 (real row-0 gathers; masked by val=0)."""
    n = cap * 128
    buf = np.zeros(n, np.int16)
    buf[: len(idx_arr)] = idx_arr.astype(np.int16)
    blk = buf.reshape(n // 16, 16).T  # [16, n/16]
    return np.tile(blk, (8, 1))       # [128, n/16]


CLS_LIST = (32, 64, 96, 128)


def _pow2w(span):
    for w in CLS_LIST:
        if span <= w:
            return w
    return 128


def _pos_win_loc(col):
    """Unified [A | B] layout position, window and in-window loc."""
    c2 = col // SHARD
    rr = col % SHARD
    pos_a = c2 * HA + rr
    pos_b = c2 * HB + (rr - HA)
    in_a = rr < HA
    win = np.where(in_a, pos_a >> 15, 2 + (pos_b >> 15)).astype(np.int64)
    loc = np.where(in_a, pos_a, pos_b) & 32767
    return win, loc


def _hop_meta(edge_row, edge_col, edge_val, keep=None):
    """Metadata for one hop: per-(sg,win) 128-edge blocks sorted by dest
    row, split into banded pieces; gather idx tables; strip scol/sval."""
    if keep is not None:
        edge_row = edge_row[keep]
        edge_col = edge_col[keep]
        edge_val = edge_val[keep]
    owner = edge_row // SHARD
    win_all, loc_all = _pos_win_loc(edge_col.astype(np.int64))
    per_core = []
    for c in range(NCORES):
        m = owner == c
        r = edge_row[m].astype(np.int64) - c * SHARD
        sg = r >> 10
        rsg = r & 1023
        win = win_all[m]
        loc = loc_all[m]
        val = edge_val[m]
        order = np.lexsort((rsg, win, sg))
        col = edge_col[m].astype(np.int64)
        per_core.append(dict(sg=sg[order], win=win[order], rsg=rsg[order],
                             loc=loc[order], val=val[order],
                             col=col[order]))

    run_counts = np.zeros((NCORES, NSG * NWIN), np.int64)
    run_starts = np.zeros((NCORES, NSG * NWIN + 1), np.int64)
    for c in range(NCORES):
        d = per_core[c]
        key = d["sg"] * NWIN + d["win"]
        run_counts[c] = np.bincount(key, minlength=NSG * NWIN)
        run_starts[c, 1:] = np.cumsum(run_counts[c])

    cap_blk = np.zeros(NSG * NWIN, np.int64)
    for k in range(NSG * NWIN):
        cap_blk[k] = _ceil(int(run_counts[:, k].max()), 128)

    sg_bof = []      # per sg: block offset of each win within its GROUP
    nblk01 = []
    nblk23 = []
    for s in range(NSG):
        off = [0] * NWIN
        off[1] = int(cap_blk[s * NWIN + 0])
        off[2] = 0
        off[3] = int(cap_blk[s * NWIN + 2])
        sg_bof.append(off)
        nblk01.append(off[1] + int(cap_blk[s * NWIN + 1]))
        nblk23.append(off[3] + int(cap_blk[s * NWIN + 3]))
    TOT01 = max(nblk01)
    TOT23 = max(nblk23)

    # ---- pieces: per (s,w,b), union-band splits ----
    pieces = [[] for _ in range(NSG)]
    for s in range(NSG):
        for w in range(NWIN):
            k = s * NWIN + w
            cap = int(cap_blk[k])
            for b in range(cap):
                segs = []
                for c in range(NCORES):
                    st, n = run_starts[c, k], run_counts[c, k]
                    lo, hi = b * 128, min(b * 128 + 128, int(n))
                    if lo < n:
                        segs.append(per_core[c]["rsg"][st + lo: st + hi])
                vals = np.concatenate(segs) if segs else np.zeros(1, np.int64)
                lo, hi = int(vals.min()), int(vals.max())
                p_lo = lo
                while True:
                    bound = (p_lo // 512 + 1) * 512
                    p_hi_allow = min(hi, bound - 1, p_lo + 127)
                    sel = vals[(vals >= p_lo) & (vals <= p_hi_allow)]
                    p_hi = int(sel.max())
                    W = _pow2w(p_hi - p_lo + 1)
                    b0 = max(min(p_lo, bound - W), (p_lo // 512) * 512)
                    pieces[s].append((s, w, b, b0, W, p_lo, p_hi))
                    nxt = vals[vals > p_hi_allow]
                    if len(nxt) == 0:
                        break
                    p_lo = int(nxt.min())

    cls_list = CLS_LIST
    cls_counts = np.zeros((NSG, len(cls_list)), np.int64)
    piece_meta = [[] for _ in range(NSG)]
    col_acc = 0
    cls_off = np.zeros((NSG, len(cls_list)), np.int64)
    for s in range(NSG):
        for wi, W in enumerate(cls_list):
            cls_off[s, wi] = col_acc
            nw = sum(1 for pc in pieces[s] if pc[4] == W)
            cls_counts[s, wi] = nw
            col_acc += nw
    NPIECE = col_acc

    for s in range(NSG):
        ci_ctr = [0] * len(cls_list)
        for (ss, w, b, b0, W, p_lo, p_hi) in pieces[s]:
            wi = cls_list.index(W)
            ci = ci_ctr[wi]
            ci_ctr[wi] += 1
            gcol = int(cls_off[s, wi]) + ci
            piece_meta[s].append((w, b, b0, W, wi, ci, gcol, p_lo, p_hi))

    GCOLS = int(sum(cap_blk)) * 8
    NBLKT = int(sum(cap_blk))
    gidx_arrs = []
    scol_arrs = []
    sval_arrs = []
    gcols_arrs = []
    for c in range(NCORES):
        d = per_core[c]
        gidx = np.zeros((128, GCOLS), np.int16)
        gcols = np.full(NBLKT * 128, -1, np.int64)
        scol2 = np.full((128, 2 * NPIECE), -1.0, np.float16)
        sval2 = np.zeros((128, 2 * NPIECE), np.float16)
        gcol = 0
        for s in range(NSG):
            for w in range(NWIN):
                k = s * NWIN + w
                cap = int(cap_blk[k])
                if cap == 0:
                    continue
                st, n = run_starts[c, k], run_counts[c, k]
                loc = d["loc"][st: st + n]
                gidx[:, gcol: gcol + cap * 8] = _pack_idx(loc, cap)
                e0 = gcol * 16
                gcols[e0: e0 + int(n)] = d["col"][st: st + n]
                gcol += cap * 8
        for s in range(NSG):
            for (w, b, b0, W, wi, ci, gc, p_lo, p_hi) in piece_meta[s]:
                k = s * NWIN + w
                st, n = run_starts[c, k], run_counts[c, k]
                lo, hi = b * 128, min(b * 128 + 128, int(n))
                scol = np.zeros(128, np.float16)
                vcol = np.zeros(128, np.float16)
                if lo < n:
                    seg_r = d["rsg"][st + lo: st + hi]
                    seg_v = d["val"][st + lo: st + hi]
                    sel = (seg_r >= p_lo) & (seg_r <= p_hi)
                    scol[: hi - lo][sel] = (seg_r[sel] - b0).astype(np.float16)
                    vcol[: hi - lo][sel] = seg_v[sel].astype(np.float16)
                scol2[:, 2 * gc] = scol
                scol2[:, 2 * gc + 1] = scol
                sval2[:, 2 * gc] = vcol
                sval2[:, 2 * gc + 1] = vcol
        gidx_arrs.append(gidx)
        scol_arrs.append(scol2)
        sval_arrs.append(sval2)
        gcols_arrs.append(gcols)

    structure = (tuple(cap_blk.tolist()),
                 tuple(tuple(pm[:5]) for s in range(NSG) for pm in piece_meta[s]),
                 TOT01, TOT23, NPIECE)
    return dict(cap_blk=cap_blk, sg_bof=sg_bof, TOT01=TOT01, TOT23=TOT23,
                nblk01=nblk01, nblk23=nblk23,
                piece_meta=piece_meta, cls_counts=cls_counts,
                cls_off=cls_off, NPIECE=NPIECE, GCOLS=GCOLS,
                gidx_arrs=gidx_arrs, scol_arrs=scol_arrs,
                sval_arrs=sval_arrs, gcols_arrs=gcols_arrs,
                NBLKT=NBLKT, structure=structure)


def _prep(edge_row, edge_col, edge_val, pair_idx):
    m1 = _hop_meta(edge_row, edge_col, edge_val)
    needed = np.zeros(NPAD, bool)
    needed[pair_idx] = True
    keep = needed[edge_row]
    m2 = _hop_meta(edge_row, edge_col, edge_val, keep=keep)
    return (m1["structure"], m2["structure"]), dict(h1=m1, h2=m2)


def _build_program(structure, meta):
    import concourse.bass as bass
    import concourse.mybir as mybir
    import concourse.tile as tile
    from concourse import bacc

    f16 = mybir.dt.float16
    f32 = mybir.dt.float32
    i16 = mybir.dt.int16
    AP = bass.AP

    cls_list = CLS_LIST
    mA, mB = meta["h1"], meta["h2"]
    GX = mB["GCOLS"]
    NP2 = max(mA["NPIECE"], mB["NPIECE"])
    T01 = max(mA["TOT01"], mB["TOT01"])
    T23 = max(mA["TOT23"], mB["TOT23"])
    BLK01 = int(sum(mA["nblk01"]))
    BLK23 = int(sum(mA["nblk23"]))

    nc = bacc.Bacc(None, num_devices=NCORES, num_swdge_queues=4)
    x0sh = nc.dram_tensor("x0sh", [SHARD, D], f16, kind="ExternalInput")
    x0g01 = nc.dram_tensor("x0g01", [P, BLK01, P], f16,
                           kind="ExternalInput")
    x0g23 = nc.dram_tensor("x0g23", [P, BLK23, P], f16,
                           kind="ExternalInput")
    scolA = nc.dram_tensor("scolA", [P, 2 * mA["NPIECE"]], f16, kind="ExternalInput")
    svalA = nc.dram_tensor("svalA", [P, 2 * mA["NPIECE"]], f16, kind="ExternalInput")
    gidxB = nc.dram_tensor("gidxB", [P, mB["GCOLS"]], i16, kind="ExternalInput")
    scolB = nc.dram_tensor("scolB", [P, 2 * mB["NPIECE"]], f16, kind="ExternalInput")
    svalB = nc.dram_tensor("svalB", [P, 2 * mB["NPIECE"]], f16, kind="ExternalInput")
    w1 = nc.dram_tensor("w1", [D, D], f16, kind="ExternalInput")
    w2 = nc.dram_tensor("w2", [D, D], f16, kind="ExternalInput")
    brep1 = nc.dram_tensor("brep1", [P, D], f32, kind="ExternalInput")
    brep2 = nc.dram_tensor("brep2", [P, D], f32, kind="ExternalInput")
    xn_out = nc.dram_tensor("xn", [3, SHARD, D], f16, kind="ExternalOutput")

    gcoB = [0] * (NSG * NWIN)
    acc = 0
    for s in range(NSG):
        for w in range(NWIN):
            gcoB[s * NWIN + w] = acc
            acc += int(mB["cap_blk"][s * NWIN + w]) * 8
    o01 = [0] * (NSG + 1)
    o23 = [0] * (NSG + 1)
    for s in range(NSG):
        o01[s + 1] = o01[s] + int(mA["nblk01"][s])
        o23[s + 1] = o23[s] + int(mA["nblk23"][s])

    with tile.TileContext(nc) as tc:
        with (
            tc.tile_pool(name="const", bufs=1) as cpool,
            tc.tile_pool(name="meta", bufs=1) as mpool,
            tc.tile_pool(name="g01", bufs=3) as g01pool,
            tc.tile_pool(name="g23", bufs=2) as g23pool,
            tc.tile_pool(name="strip", bufs=2) as spool,
            tc.tile_pool(name="work", bufs=2) as wpool,
            tc.tile_pool(name="norm", bufs=2) as npool,
            tc.tile_pool(name="psy", bufs=2, space="PSUM") as psy,
            tc.tile_pool(name="psx", bufs=2, space="PSUM") as psx,
            tc.tile_pool(name="dram", bufs=1, space="DRAM") as dram,
        ):
            # constants
            iota_i = cpool.tile([P, P], mybir.dt.int32)
            nc.gpsimd.iota(iota_i, pattern=[[1, P]], base=0,
                           channel_multiplier=0)
            iota16 = cpool.tile([P, P], f16)
            nc.vector.tensor_copy(iota16, iota_i)
            eps_t = cpool.tile([P, 1], f32)
            nc.vector.memset(eps_t[:, :], 1e-24)
            w1_t = cpool.tile([P, P], f16)
            nc.sync.dma_start(out=w1_t, in_=w1[:, :])
            w2_t = cpool.tile([P, P], f16)
            nc.sync.dma_start(out=w2_t, in_=w2[:, :])
            b1_t = cpool.tile([P, P], f32)
            nc.sync.dma_start(out=b1_t, in_=brep1[:, :])
            b2_t = cpool.tile([P, P], f32)
            nc.sync.dma_start(out=b2_t, in_=brep2[:, :])
            gidx_t = mpool.tile([P, GX], i16)
            scol_t = mpool.tile([P, 2 * NP2], f16)
            nc.sync.dma_start(out=scol_t[:, : 2 * mA["NPIECE"]], in_=scolA[:, :])
            sval_t = mpool.tile([P, 2 * NP2], f16)
            nc.sync.dma_start(out=sval_t[:, : 2 * mA["NPIECE"]], in_=svalA[:, :])

            # internal DRAM
            xsh1a = dram.tile([HA, D], f16)
            xsh1b = dram.tile([HB, D], f16)
            xg1a = dram.tile([NA, D], f16, addr_space="Shared")
            xg1b = dram.tile([NB, D], f16, addr_space="Shared")

            def emit_ag(part):
                src_, dst = [(xsh1a, xg1a), (xsh1b, xg1b)][part]
                nc.gpsimd.collective_compute(
                    "AllGather", mybir.AluOpType.bypass,
                    replica_groups=[list(range(NCORES))],
                    ins=[src_.opt()], outs=[dst.opt()],
                )

            def gather_chunks(m, srcs, s, t01, t23, wins, qload):
                cap_blk = m["cap_blk"]
                sg_bof = m["sg_bof"]
                CH = 22
                for w in wins:
                    k = s * NWIN + w
                    cap = int(cap_blk[k])
                    if cap == 0:
                        continue
                    gbuf = t01 if w < 2 else t23
                    for lo in range(0, cap, CH):
                        ln = min(CH, cap - lo)
                        q = qload.index(min(qload))
                        qload[q] += ln
                        bo = sg_bof[s][w] + lo
                        co = gcoB[k] + lo * 8
                        nc.gpsimd.dma_gather(
                            gbuf[:, bo: bo + ln, :],
                            srcs[w],
                            gidx_t[:, co: co + ln * 8],
                            num_idxs=ln * 128, num_idxs_reg=ln * 128,
                            elem_size=P, single_packet=False,
                            queue_num=q,
                        )

            def norm_rows(xin, ntl, dst_ap):
                """xin: [P, ntl, P] f16 SBUF; l2-normalize rows in place and
                DMA (f16) to dst_ap (DRAM, (c p) d layout)."""
                sqf = npool.tile([P, P], f16, tag="sqf")
                rs = npool.tile([P, SG_TILES], f32, tag="rs")
                for t in range(ntl):
                    nc.scalar.activation(
                        sqf[:, :], xin[:, t, :],
                        mybir.ActivationFunctionType.Square,
                        accum_out=rs[:, t: t + 1])
                nrm = npool.tile([P, SG_TILES], f32, tag="nrm")
                nc.scalar.activation(nrm[:, :ntl], rs[:, :ntl],
                                     mybir.ActivationFunctionType.Sqrt,
                                     bias=eps_t[:, :1])
                rinv2 = npool.tile([P, SG_TILES, 2], f16, tag="rinv2")
                with nc.allow_low_precision(reason="f16 rinv; 5e-4 ok"):
                    nc.vector.reciprocal(rinv2[:, :ntl, 0], nrm[:, :ntl])
                    nc.vector.reciprocal(rinv2[:, :ntl, 1], nrm[:, :ntl])
                xv = xin[:, :ntl, :]
                xi4 = AP(xv.tensor, xv.offset,
                         [xv.ap[0], [P, ntl], [2, 64], [1, 2]])
                ri = rinv2[:, :ntl, :]
                ri4 = AP(ri.tensor, ri.offset,
                         [ri.ap[0], [2, ntl], [0, 64], [1, 2]])
                nc.vector.tensor_tensor(out=xi4, in0=xi4, in1=ri4,
                                        op=mybir.AluOpType.mult)
                nc.sync.dma_start(
                    out=dst_ap.rearrange("(c p) d -> p c d", p=P),
                    in_=xin[:, :ntl, :])

            def pair0_stage():
                for s in range(NSG):
                    ntl = min(SG_TILES, NTILE - s * SG_TILES)
                    r0 = s * SGR
                    x0t = npool.tile([P, SG_TILES, P], f16, tag="x0t")
                    nc.sync.dma_start(
                        out=x0t[:, :ntl, :],
                        in_=x0sh[r0: r0 + ntl * 128, :].rearrange(
                            "(c p) d -> p c d", p=P))
                    norm_rows(x0t, ntl, xn_out[0, r0: r0 + ntl * 128, :])

            def compute_sg(m, s, t01, t23, w_t, b_t, xsh_fn, hop):
                ntl = min(SG_TILES, NTILE - s * SG_TILES)
                nrow = ntl * 128
                pm = m["piece_meta"][s]
                sg_bof = m["sg_bof"]
                strips = {}
                for wi, W in enumerate(cls_list):
                    ncl = int(m["cls_counts"][s, wi])
                    if ncl == 0:
                        continue
                    st = spool.tile([P, max(ncl, 1), W], f16,
                                    tag=f"str{W}")
                    c0 = int(m["cls_off"][s, wi])
                    stv = st[:, :ncl, :]
                    st4 = AP(stv.tensor, stv.offset,
                             [stv.ap[0], [W, ncl], [2, W // 2], [1, 2]])
                    io = iota16[:, :]
                    io4 = AP(io.tensor, io.offset,
                             [io.ap[0], [0, ncl], [2, W // 2], [1, 2]])
                    sc = scol_t[:, 2 * c0: 2 * (c0 + ncl)]
                    sc4 = AP(sc.tensor, sc.offset,
                             [sc.ap[0], [2, ncl], [0, W // 2], [1, 2]])
                    sv = sval_t[:, 2 * c0: 2 * (c0 + ncl)]
                    sv4 = AP(sv.tensor, sv.offset,
                             [sv.ap[0], [2, ncl], [0, W // 2], [1, 2]])
                    nc.vector.tensor_tensor(
                        out=st4, in0=io4, in1=sc4,
                        op=mybir.AluOpType.is_equal)
                    nc.vector.tensor_tensor(
                        out=st4, in0=st4, in1=sv4,
                        op=mybir.AluOpType.mult)
                    strips[W] = st

                y_ps = psy.tile([P, SGR], f32, space="PSUM", tag="y")
                nc.scalar.memzero(y_ps[:, :nrow])
                nmm = len(pm)
                for i, (w, b, b0, W, wi, ci, gc, p_lo, p_hi) in enumerate(pm):
                    gbuf = t01 if w < 2 else t23
                    gb = sg_bof[s][w] + b
                    nc.tensor.matmul(
                        y_ps[:, b0: b0 + W], lhsT=gbuf[:, gb, :],
                        rhs=strips[W][:, ci, :],
                        start=False, stop=(i == nmm - 1),
                        skip_group_check=True,
                    )
                yT = wpool.tile([P, SGR], f16, tag="yT")
                nc.scalar.copy(yT[:, :nrow], y_ps[:, :nrow])
                x_ps = psx.tile([P, SGR], f32, space="PSUM", tag="x")
                for t in range(ntl):
                    nc.tensor.matmul(x_ps[:, t * 128: (t + 1) * 128],
                                     lhsT=yT[:, t * 128: (t + 1) * 128],
                                     rhs=w_t, start=True, stop=True)
                xb = npool.tile([P, SG_TILES, P], f16, tag="xb")
                bt = b_t[:, :]
                bt4 = AP(bt.tensor, bt.offset, [bt.ap[0], [0, ntl], [1, P]])
                xp = x_ps[:, :nrow]
                xp3 = AP(xp.tensor, xp.offset,
                         [xp.ap[0], [P, ntl], [1, P]])
                nc.vector.tensor_tensor(
                    out=xb[:, :ntl, :], in0=xp3, in1=bt4,
                    op=mybir.AluOpType.add)
                r0 = s * SGR
                if xsh_fn is not None:
                    nc.sync.dma_start(
                        out=xsh_fn(s, ntl).rearrange(
                            "(c p) d -> p c d", p=P),
                        in_=xb[:, :ntl, :])
                norm_rows(xb, ntl, xn_out[hop, r0: r0 + ntl * 128, :])

            def graph_hop1(m, w_t, b_t, xsh_fn, hop, post_gather=None):
                for s in range(NSG):
                    n01 = o01[s + 1] - o01[s]
                    n23 = o23[s + 1] - o23[s]
                    t01 = g01pool.tile([P, T01, P], f16, tag="g01",
                                       name="t01")
                    t23 = g23pool.tile([P, T23, P], f16, tag="g23",
                                       name="t23")
                    if n01:
                        nc.sync.dma_start(
                            out=t01[:, :n01, :],
                            in_=x0g01[:, o01[s]: o01[s + 1], :])
                    if n23:
                        nc.sync.dma_start(
                            out=t23[:, :n23, :],
                            in_=x0g23[:, o23[s]: o23[s + 1], :])
                    if post_gather is not None:
                        post_gather(s)
                    compute_sg(m, s, t01, t23, w_t, b_t, xsh_fn, hop)

            def graph_hop2(m, srcs, w_t, b_t, hop):
                """hop 2: AGb fired first (CC waits internally); win-0/1
                gathers (xg1a, ready right after h1) prefetch 3 sgs deep;
                pair0 fills the AGb window; win-2/3 gathers stream with
                compute."""
                qload = [0, 0, 0, 0]
                t01s = {}
                t23s = {}
                npre = min(3, NSG)
                for s in range(npre):
                    t01s[s] = g01pool.tile([P, T01, P], f16, tag="g01",
                                           name="t01")
                    gather_chunks(m, srcs, s, t01s[s], None, [0, 1], qload)
                    if s == 0:
                        emit_ag(1)
                pair0_stage()
                for s in range(NSG):
                    t23s[s] = g23pool.tile([P, T23, P], f16, tag="g23",
                                           name="t23")
                    gather_chunks(m, srcs, s, None, t23s[s], [2, 3], qload)
                    compute_sg(m, s, t01s[s], t23s[s], w_t, b_t, None, hop)
                    if s + npre < NSG:
                        t01s[s + npre] = g01pool.tile(
                            [P, T01, P], f16, tag="g01", name="t01")
                        gather_chunks(m, srcs, s + npre, t01s[s + npre],
                                      None, [0, 1], qload)

            def xsh1_fn(s, ntl):
                if s < 6:
                    r = s * SGR
                    return xsh1a[r: r + ntl * 128, :]
                r = (s - 6) * SGR
                return xsh1b[r: r + ntl * 128, :]

            srcs2 = [xg1a[0: WIN, :], xg1a[WIN: NA, :],
                     xg1b[0: WIN, :], xg1b[WIN: NB, :]]

            def h1_hook(s):
                # trigger the A-half AllGather once its inputs exist
                if s == 6:
                    emit_ag(0)
                elif s == 8:
                    nc.sync.dma_start(out=gidx_t[:, :], in_=gidxB[:, :])

            stages = os.environ.get(
                "BASS_GNN_STAGES", "p0,h1,h2").split(",")
            if "h1" in stages:
                graph_hop1(mA, w1_t, b1_t, xsh1_fn, 1,
                           post_gather=h1_hook)
            # overwrite metadata SBUF with hop-2 tables (Tile orders these
            # after hop-1's last reads)
            nc.sync.dma_start(out=scol_t[:, : 2 * mB["NPIECE"]],
                              in_=scolB[:, :])
            nc.sync.dma_start(out=sval_t[:, : 2 * mB["NPIECE"]],
                              in_=svalB[:, :])
            if "h2" in stages:
                graph_hop2(mB, srcs2, w2_t, b2_t, 2)
            elif "p0" in stages:
                pair0_stage()

    nc.compile()
    return nc


def _install_ntff_shim():
    """Provide antenv.axon_hooks (missing on this image) so trace=True can
    capture NTFF profiles through the axon .so."""
    import types
    if "antenv.axon_hooks" in sys.modules:
        return
    mod = types.ModuleType("antenv.axon_hooks")
    mod._hook = None

    def set_axon_ntff_profile_hook(h):
        mod._hook = h

    def get_axon_ntff_profile_hook():
        return mod._hook

    mod.set_axon_ntff_profile_hook = set_axon_ntff_profile_hook
    mod.get_axon_ntff_profile_hook = get_axon_ntff_profile_hook
    sys.modules["antenv.axon_hooks"] = mod
    try:
        from trn_agent_boot.trn_boot import _ntff_profile_via_ctypes
        mod._hook = _ntff_profile_via_ctypes("/opt/axon/libaxon_pjrt.so")
    except Exception:
        mod._hook = None


def _interleave(x):
    """[NPAD, D] global row order -> [A | B1 | B2] gather layout."""
    xs = x.reshape(NCORES, SHARD, -1)
    a = xs[:, :HA].reshape(NA, -1)
    b1 = xs[:, HA: HA + HB1].reshape(NB1, -1)
    b2 = xs[:, HA + HB1:].reshape(NB2, -1)
    return np.concatenate([a, b1, b2], axis=0)


def kernel(node_emb, attri_emb, W1, b1, W2, b2, edge_val,
           edge_row, edge_col, pos_src, pos_dst, neg_src, neg_dst):
    global LAST_RESULTS
    _install_ntff_shim()
    from concourse.bass_utils import run_bass_kernel_spmd

    pair_idx = np.concatenate([pos_src, pos_dst, neg_src, neg_dst]).astype(
        np.int64)
    structure, meta = _prep(edge_row.astype(np.int64),
                            edge_col.astype(np.int64), edge_val, pair_idx)

    import time as _time
    key = (structure, os.environ.get("BASS_GNN_STAGES", ""))
    if key in _CACHE:
        nc = _CACHE[key]
    else:
        t0 = _time.time()
        nc = _build_program(structure, meta)
        print(f"[kernel] build+schedule: {_time.time() - t0:.1f}s, "
              f"{len(nc.inst_map)} instructions", flush=True)
        _CACHE[key] = nc

    x0 = np.concatenate([node_emb, attri_emb], axis=0).astype(np.float32)
    x0p = np.zeros((NPAD, D), np.float32)
    x0p[:N] = x0
    x0f16 = x0p.astype(np.float16)

    mA, mB = meta["h1"], meta["h2"]
    in_maps = []
    for c in range(NCORES):
        gcols = mA["gcols_arrs"][c]
        rows = np.zeros((mA["NBLKT"] * 128, D), np.float16)
        sel = gcols >= 0
        rows[sel] = x0f16[gcols[sel]]
        blocks = rows.reshape(mA["NBLKT"], 128, D)
        b01 = []
        b23 = []
        goff = 0
        for s in range(NSG):
            n01 = int(mA["nblk01"][s])
            n23 = int(mA["nblk23"][s])
            b01.append(blocks[goff: goff + n01])
            b23.append(blocks[goff + n01: goff + n01 + n23])
            goff += n01 + n23
        x0g01 = np.ascontiguousarray(
            np.concatenate(b01, 0).transpose(1, 0, 2))
        x0g23 = np.ascontiguousarray(
            np.concatenate(b23, 0).transpose(1, 0, 2))
        in_maps.append({
            "x0sh": x0f16[c * SHARD: (c + 1) * SHARD],
            "x0g01": x0g01,
            "x0g23": x0g23,
            "scolA": mA["scol_arrs"][c],
            "svalA": mA["sval_arrs"][c],
            "gidxB": mB["gidx_arrs"][c],
            "scolB": mB["scol_arrs"][c],
            "svalB": mB["sval_arrs"][c],
            "w1": W1.astype(np.float16),
            "w2": W2.astype(np.float16),
            "brep1": np.broadcast_to(
                b1.astype(np.float32)[None, :], (P, D)).copy(),
            "brep2": np.broadcast_to(
                b2.astype(np.float32)[None, :], (P, D)).copy(),
        })

    trace = os.environ.get("BASS_GNN_TRACE", "0") == "1"
    t0 = _time.time()
    res = run_bass_kernel_spmd(nc, in_maps, core_ids=list(range(NCORES)),
                               trace=trace)
    print(f"[kernel] compile+run: {_time.time() - t0:.1f}s", flush=True)
    LAST_RESULTS = res

    # ---- host assembly: index normalized tables per hop ----
    xn_full = np.empty((3, NPAD, D), np.float32)
    for c in range(NCORES):
        xn_full[:, c * SHARD: (c + 1) * SHARD] = \
            res.results[c]["xn"].astype(np.float32)
    out = np.empty((4, 3, E_PAIR, D), np.float32)
    for st, idx in enumerate((pos_src, pos_dst, neg_src, neg_dst)):
        idx64 = idx.astype(np.int64)
        for h in range(3):
            out[st, h] = xn_full[h, idx64]
    return out


# revision 23
# speedup vs baseline: 1.0399x; 1.0399x over previous
"""GCN message-passing kernel for trn2 (8 NeuronCores, SPMD + split AllGather).

v7 strategy (banded strips + 3-way AG + hop-2 dest filtering):
  - Shard the N=100352 (padded) node dim across 8 cores (12544 rows each).
  - Unified 4-window source layout: x0 is pre-permuted on host into
    [A | B1 | B2] segments matching the three AllGather outputs, so hop-1
    and hop-2 use the same (win, loc) addressing.
  - Per sg (1024 dest rows), segment-sum runs as PE matmuls into a
    [128, 1024] fp32 PSUM accumulator: per 128-edge block a narrow banded
    one-hot routing strip (width 32/64/128 covering the block's dest-row
    span) is built on DVE; pieces split at PSUM 512-col bank boundaries.
  - SWDGE descriptor generation on the Pool engine is the bottleneck
    (~2.2ns/edge, serial), so: hop-2 keeps only edges whose dest row is
    referenced by a pair index (-13.5%), the AllGather is split 3 ways
    (sgs 0-5 / 6-9 / 10-12) triggered at h1 sg7 / sg11 / end so hop-2
    windows 0-2 are gatherable immediately after hop-1, and hop-2
    prefetches two sgs of win-0/1/2 gathers before pair0 fills the
    AGb2 window.
  - x_new = (A x) @ W + b per tile by swapping matmul operand roles; norm
    squares on the Scalar engine; everything fp16 2x-mode where possible.
  - Pair streams assembled on host from per-hop normalized tables (fp16).
"""
import os
import sys

sys.path.insert(0, "/opt/trn_rl_repo")

import numpy as np

N = 100000
D = 128
NCORES = 8
SHARD = 12544            # 98 tiles of 128
NTILE = SHARD // 128     # 98
NPAD = SHARD * NCORES    # 100352
WIN = 32768
NWIN = 4
SG_TILES = 8
NSG = (NTILE + SG_TILES - 1) // SG_TILES  # 13
SGR = SG_TILES * 128     # 1024 rows per sg
HA = 6144                # sgs 0-5  -> segment A
HB = SHARD - HA          # sgs 6-12 -> segment B (6400 rows)
NA = HA * NCORES         # 49152
NB = HB * NCORES         # 51200
E_PAIR = 50000
P = 128

_CACHE = {}
LAST_RESULTS = None  # BassKernelResults of the most recent run (for test.py)


def _ceil(a, b):
    return -(-a // b)


def _pack_idx(idx_arr, cap):
    """Pack idx list (len<=cap*128, int) to the [128, cap*8] wrapped+replicated
    int16 layout. Pads with 0 (real row-0 gathers; masked by val=0)."""
    n = cap * 128
    buf = np.zeros(n, np.int16)
    buf[: len(idx_arr)] = idx_arr.astype(np.int16)
    blk = buf.reshape(n // 16, 16).T  # [16, n/16]
    return np.tile(blk, (8, 1))       # [128, n/16]


CLS_LIST = (32, 64, 96, 128)


def _pow2w(span):
    for w in CLS_LIST:
        if span <= w:
            return w
    return 128


def _pos_win_loc(col):
    """Unified [A | B] layout position, window and in-window loc."""
    c2 = col // SHARD
    rr = col % SHARD
    pos_a = c2 * HA + rr
    pos_b = c2 * HB + (rr - HA)
    in_a = rr < HA
    win = np.where(in_a, pos_a >> 15, 2 + (pos_b >> 15)).astype(np.int64)
    loc = np.where(in_a, pos_a, pos_b) & 32767
    return win, loc


def _hop_meta(edge_row, edge_col, edge_val, keep=None):
    """Metadata for one hop: per-(sg,win) 128-edge blocks sorted by dest
    row, split into banded pieces; gather idx tables; strip scol/sval."""
    if keep is not None:
        edge_row = edge_row[keep]
        edge_col = edge_col[keep]
        edge_val = edge_val[keep]
    owner = edge_row // SHARD
    win_all, loc_all = _pos_win_loc(edge_col.astype(np.int64))
    per_core = []
    for c in range(NCORES):
        m = owner == c
        r = edge_row[m].astype(np.int64) - c * SHARD
        sg = r >> 10
        rsg = r & 1023
        win = win_all[m]
        loc = loc_all[m]
        val = edge_val[m]
        order = np.lexsort((rsg, win, sg))
        col = edge_col[m].astype(np.int64)
        per_core.append(dict(sg=sg[order], win=win[order], rsg=rsg[order],
                             loc=loc[order], val=val[order],
                             col=col[order]))

    run_counts = np.zeros((NCORES, NSG * NWIN), np.int64)
    run_starts = np.zeros((NCORES, NSG * NWIN + 1), np.int64)
    for c in range(NCORES):
        d = per_core[c]
        key = d["sg"] * NWIN + d["win"]
        run_counts[c] = np.bincount(key, minlength=NSG * NWIN)
        run_starts[c, 1:] = np.cumsum(run_counts[c])

    cap_blk = np.zeros(NSG * NWIN, np.int64)
    for k in range(NSG * NWIN):
        cap_blk[k] = _ceil(int(run_counts[:, k].max()), 128)

    sg_bof = []      # per sg: block offset of each win within its GROUP
    nblk01 = []
    nblk23 = []
    for s in range(NSG):
        off = [0] * NWIN
        off[1] = int(cap_blk[s * NWIN + 0])
        off[2] = 0
        off[3] = int(cap_blk[s * NWIN + 2])
        sg_bof.append(off)
        nblk01.append(off[1] + int(cap_blk[s * NWIN + 1]))
        nblk23.append(off[3] + int(cap_blk[s * NWIN + 3]))
    TOT01 = max(nblk01)
    TOT23 = max(nblk23)

    # ---- pieces: per (s,w,b), union-band splits ----
    pieces = [[] for _ in range(NSG)]
    for s in range(NSG):
        for w in range(NWIN):
            k = s * NWIN + w
            cap = int(cap_blk[k])
            for b in range(cap):
                segs = []
                for c in range(NCORES):
                    st, n = run_starts[c, k], run_counts[c, k]
                    lo, hi = b * 128, min(b * 128 + 128, int(n))
                    if lo < n:
                        segs.append(per_core[c]["rsg"][st + lo: st + hi])
                vals = np.concatenate(segs) if segs else np.zeros(1, np.int64)
                lo, hi = int(vals.min()), int(vals.max())
                p_lo = lo
                while True:
                    bound = (p_lo // 512 + 1) * 512
                    p_hi_allow = min(hi, bound - 1, p_lo + 127)
                    sel = vals[(vals >= p_lo) & (vals <= p_hi_allow)]
                    p_hi = int(sel.max())
                    W = _pow2w(p_hi - p_lo + 1)
                    b0 = max(min(p_lo, bound - W), (p_lo // 512) * 512)
                    pieces[s].append((s, w, b, b0, W, p_lo, p_hi))
                    nxt = vals[vals > p_hi_allow]
                    if len(nxt) == 0:
                        break
                    p_lo = int(nxt.min())

    cls_list = CLS_LIST
    cls_counts = np.zeros((NSG, len(cls_list)), np.int64)
    piece_meta = [[] for _ in range(NSG)]
    col_acc = 0
    cls_off = np.zeros((NSG, len(cls_list)), np.int64)
    for s in range(NSG):
        for wi, W in enumerate(cls_list):
            cls_off[s, wi] = col_acc
            nw = sum(1 for pc in pieces[s] if pc[4] == W)
            cls_counts[s, wi] = nw
            col_acc += nw
    NPIECE = col_acc

    for s in range(NSG):
        ci_ctr = [0] * len(cls_list)
        for (ss, w, b, b0, W, p_lo, p_hi) in pieces[s]:
            wi = cls_list.index(W)
            ci = ci_ctr[wi]
            ci_ctr[wi] += 1
            gcol = int(cls_off[s, wi]) + ci
            piece_meta[s].append((w, b, b0, W, wi, ci, gcol, p_lo, p_hi))

    GCOLS = int(sum(cap_blk)) * 8
    NBLKT = int(sum(cap_blk))
    gidx_arrs = []
    scol_arrs = []
    sval_arrs = []
    gcols_arrs = []
    for c in range(NCORES):
        d = per_core[c]
        gidx = np.zeros((128, GCOLS), np.int16)
        gcols = np.full(NBLKT * 128, -1, np.int64)
        scol2 = np.full((128, 2 * NPIECE), -1.0, np.float16)
        sval2 = np.zeros((128, 2 * NPIECE), np.float16)
        gcol = 0
        for s in range(NSG):
            for w in range(NWIN):
                k = s * NWIN + w
                cap = int(cap_blk[k])
                if cap == 0:
                    continue
                st, n = run_starts[c, k], run_counts[c, k]
                loc = d["loc"][st: st + n]
                gidx[:, gcol: gcol + cap * 8] = _pack_idx(loc, cap)
                e0 = gcol * 16
                gcols[e0: e0 + int(n)] = d["col"][st: st + n]
                gcol += cap * 8
        for s in range(NSG):
            for (w, b, b0, W, wi, ci, gc, p_lo, p_hi) in piece_meta[s]:
                k = s * NWIN + w
                st, n = run_starts[c, k], run_counts[c, k]
                lo, hi = b * 128, min(b * 128 + 128, int(n))
                scol = np.zeros(128, np.float16)
                vcol = np.zeros(128, np.float16)
                if lo < n:
                    seg_r = d["rsg"][st + lo: st + hi]
                    seg_v = d["val"][st + lo: st + hi]
                    sel = (seg_r >= p_lo) & (seg_r <= p_hi)
                    scol[: hi - lo][sel] = (seg_r[sel] - b0).astype(np.float16)
                    vcol[: hi - lo][sel] = seg_v[sel].astype(np.float16)
                scol2[:, 2 * gc] = scol
                scol2[:, 2 * gc + 1] = scol
                sval2[:, 2 * gc] = vcol
                sval2[:, 2 * gc + 1] = vcol
        gidx_arrs.append(gidx)
        scol_arrs.append(scol2)
        sval_arrs.append(sval2)
        gcols_arrs.append(gcols)

    structure = (tuple(cap_blk.tolist()),
                 tuple(tuple(pm[:5]) for s in range(NSG) for pm in piece_meta[s]),
                 TOT01, TOT23, NPIECE)
    return dict(cap_blk=cap_blk, sg_bof=sg_bof, TOT01=TOT01, TOT23=TOT23,
                nblk01=nblk01, nblk23=nblk23,
                piece_meta=piece_meta, cls_counts=cls_counts,
                cls_off=cls_off, NPIECE=NPIECE, GCOLS=GCOLS,
                gidx_arrs=gidx_arrs, scol_arrs=scol_arrs,
                sval_arrs=sval_arrs, gcols_arrs=gcols_arrs,
                NBLKT=NBLKT, structure=structure)


def _prep(edge_row, edge_col, edge_val, pair_idx):
    m1 = _hop_meta(edge_row, edge_col, edge_val)
    needed = np.zeros(NPAD, bool)
    needed[pair_idx] = True
    keep = needed[edge_row]
    m2 = _hop_meta(edge_row, edge_col, edge_val, keep=keep)
    return (m1["structure"], m2["structure"]), dict(h1=m1, h2=m2)


def _build_program(structure, meta):
    import concourse.bass as bass
    import concourse.mybir as mybir
    import concourse.tile as tile
    from concourse import bacc

    f16 = mybir.dt.float16
    f32 = mybir.dt.float32
    i16 = mybir.dt.int16
    AP = bass.AP

    cls_list = CLS_LIST
    mA, mB = meta["h1"], meta["h2"]
    GX = mB["GCOLS"]
    NP2 = max(mA["NPIECE"], mB["NPIECE"])
    T01 = max(mA["TOT01"], mB["TOT01"])
    T23 = max(mA["TOT23"], mB["TOT23"])
    BLK01 = int(sum(mA["nblk01"]))
    BLK23 = int(sum(mA["nblk23"]))

    nc = bacc.Bacc(None, num_devices=NCORES, num_swdge_queues=4)
    x0sh = nc.dram_tensor("x0sh", [SHARD, D], f16, kind="ExternalInput")
    x0g01 = nc.dram_tensor("x0g01", [P, BLK01, P], f16,
                           kind="ExternalInput")
    x0g23 = nc.dram_tensor("x0g23", [P, BLK23, P], f16,
                           kind="ExternalInput")
    scolA = nc.dram_tensor("scolA", [P, 2 * mA["NPIECE"]], f16, kind="ExternalInput")
    svalA = nc.dram_tensor("svalA", [P, 2 * mA["NPIECE"]], f16, kind="ExternalInput")
    gidxB = nc.dram_tensor("gidxB", [P, mB["GCOLS"]], i16, kind="ExternalInput")
    scolB = nc.dram_tensor("scolB", [P, 2 * mB["NPIECE"]], f16, kind="ExternalInput")
    svalB = nc.dram_tensor("svalB", [P, 2 * mB["NPIECE"]], f16, kind="ExternalInput")
    w1 = nc.dram_tensor("w1", [D, D], f16, kind="ExternalInput")
    w2 = nc.dram_tensor("w2", [D, D], f16, kind="ExternalInput")
    brep1 = nc.dram_tensor("brep1", [P, D], f32, kind="ExternalInput")
    brep2 = nc.dram_tensor("brep2", [P, D], f32, kind="ExternalInput")
    xn_out = nc.dram_tensor("xn", [3, SHARD, D], f16, kind="ExternalOutput")

    gcoB = [0] * (NSG * NWIN)
    acc = 0
    for s in range(NSG):
        for w in range(NWIN):
            gcoB[s * NWIN + w] = acc
            acc += int(mB["cap_blk"][s * NWIN + w]) * 8
    o01 = [0] * (NSG + 1)
    o23 = [0] * (NSG + 1)
    for s in range(NSG):
        o01[s + 1] = o01[s] + int(mA["nblk01"][s])
        o23[s + 1] = o23[s] + int(mA["nblk23"][s])

    with tile.TileContext(nc) as tc:
        with (
            tc.tile_pool(name="const", bufs=1) as cpool,
            tc.tile_pool(name="meta", bufs=1) as mpool,
            tc.tile_pool(name="g01", bufs=3) as g01pool,
            tc.tile_pool(name="g23", bufs=2) as g23pool,
            tc.tile_pool(name="strip", bufs=2) as spool,
            tc.tile_pool(name="work", bufs=2) as wpool,
            tc.tile_pool(name="norm", bufs=2) as npool,
            tc.tile_pool(name="psy", bufs=2, space="PSUM") as psy,
            tc.tile_pool(name="psx", bufs=2, space="PSUM") as psx,
            tc.tile_pool(name="dram", bufs=1, space="DRAM") as dram,
        ):
            # constants
            iota_i = cpool.tile([P, P], mybir.dt.int32)
            nc.gpsimd.iota(iota_i, pattern=[[1, P]], base=0,
                           channel_multiplier=0)
            iota16 = cpool.tile([P, P], f16)
            nc.vector.tensor_copy(iota16, iota_i)
            eps_t = cpool.tile([P, 1], f32)
            nc.vector.memset(eps_t[:, :], 1e-24)
            w1_t = cpool.tile([P, P], f16)
            nc.sync.dma_start(out=w1_t, in_=w1[:, :])
            w2_t = cpool.tile([P, P], f16)
            nc.sync.dma_start(out=w2_t, in_=w2[:, :])
            b1_t = cpool.tile([P, P], f32)
            nc.sync.dma_start(out=b1_t, in_=brep1[:, :])
            b2_t = cpool.tile([P, P], f32)
            nc.sync.dma_start(out=b2_t, in_=brep2[:, :])
            gidx_t = mpool.tile([P, GX], i16)
            nc.sync.dma_start(out=gidx_t[:, :], in_=gidxB[:, :])
            scol_t = mpool.tile([P, 2 * NP2], f16)
            nc.sync.dma_start(out=scol_t[:, : 2 * mA["NPIECE"]], in_=scolA[:, :])
            sval_t = mpool.tile([P, 2 * NP2], f16)
            nc.sync.dma_start(out=sval_t[:, : 2 * mA["NPIECE"]], in_=svalA[:, :])

            # internal DRAM
            xsh1a = dram.tile([HA, D], f16)
            xsh1b = dram.tile([HB, D], f16)
            xg1a = dram.tile([NA, D], f16, addr_space="Shared")
            xg1b = dram.tile([NB, D], f16, addr_space="Shared")

            def emit_ag(part):
                src_, dst = [(xsh1a, xg1a), (xsh1b, xg1b)][part]
                nc.gpsimd.collective_compute(
                    "AllGather", mybir.AluOpType.bypass,
                    replica_groups=[list(range(NCORES))],
                    ins=[src_.opt()], outs=[dst.opt()],
                )

            def gather_chunks(m, srcs, s, t01, t23, wins, qload):
                cap_blk = m["cap_blk"]
                sg_bof = m["sg_bof"]
                CH = 22
                for w in wins:
                    k = s * NWIN + w
                    cap = int(cap_blk[k])
                    if cap == 0:
                        continue
                    gbuf = t01 if w < 2 else t23
                    for lo in range(0, cap, CH):
                        ln = min(CH, cap - lo)
                        q = qload.index(min(qload))
                        qload[q] += ln
                        bo = sg_bof[s][w] + lo
                        co = gcoB[k] + lo * 8
                        nc.gpsimd.dma_gather(
                            gbuf[:, bo: bo + ln, :],
                            srcs[w],
                            gidx_t[:, co: co + ln * 8],
                            num_idxs=ln * 128, num_idxs_reg=ln * 128,
                            elem_size=P, single_packet=False,
                            queue_num=q,
                        )

            def norm_rows(xin, ntl, dst_ap):
                """xin: [P, ntl, P] f16 SBUF; l2-normalize rows in place and
                DMA (f16) to dst_ap (DRAM, (c p) d layout)."""
                sqf = npool.tile([P, P], f16, tag="sqf")
                rs = npool.tile([P, SG_TILES], f32, tag="rs")
                for t in range(ntl):
                    nc.scalar.activation(
                        sqf[:, :], xin[:, t, :],
                        mybir.ActivationFunctionType.Square,
                        accum_out=rs[:, t: t + 1])
                nrm = npool.tile([P, SG_TILES], f32, tag="nrm")
                nc.scalar.activation(nrm[:, :ntl], rs[:, :ntl],
                                     mybir.ActivationFunctionType.Sqrt,
                                     bias=eps_t[:, :1])
                rinv2 = npool.tile([P, SG_TILES, 2], f16, tag="rinv2")
                with nc.allow_low_precision(reason="f16 rinv; 5e-4 ok"):
                    nc.vector.reciprocal(rinv2[:, :ntl, 0], nrm[:, :ntl])
                    nc.vector.reciprocal(rinv2[:, :ntl, 1], nrm[:, :ntl])
                xv = xin[:, :ntl, :]
                xi4 = AP(xv.tensor, xv.offset,
                         [xv.ap[0], [P, ntl], [2, 64], [1, 2]])
                ri = rinv2[:, :ntl, :]
                ri4 = AP(ri.tensor, ri.offset,
                         [ri.ap[0], [2, ntl], [0, 64], [1, 2]])
                nc.vector.tensor_tensor(out=xi4, in0=xi4, in1=ri4,
                                        op=mybir.AluOpType.mult)
                nc.sync.dma_start(
                    out=dst_ap.rearrange("(c p) d -> p c d", p=P),
                    in_=xin[:, :ntl, :])

            def pair0_stage():
                for s in range(NSG):
                    ntl = min(SG_TILES, NTILE - s * SG_TILES)
                    r0 = s * SGR
                    x0t = npool.tile([P, SG_TILES, P], f16, tag="x0t")
                    nc.sync.dma_start(
                        out=x0t[:, :ntl, :],
                        in_=x0sh[r0: r0 + ntl * 128, :].rearrange(
                            "(c p) d -> p c d", p=P))
                    norm_rows(x0t, ntl, xn_out[0, r0: r0 + ntl * 128, :])

            def compute_sg(m, s, t01, t23, w_t, b_t, xsh_fn, hop):
                ntl = min(SG_TILES, NTILE - s * SG_TILES)
                nrow = ntl * 128
                pm = m["piece_meta"][s]
                sg_bof = m["sg_bof"]
                strips = {}
                for wi, W in enumerate(cls_list):
                    ncl = int(m["cls_counts"][s, wi])
                    if ncl == 0:
                        continue
                    st = spool.tile([P, max(ncl, 1), W], f16,
                                    tag=f"str{W}")
                    c0 = int(m["cls_off"][s, wi])
                    stv = st[:, :ncl, :]
                    st4 = AP(stv.tensor, stv.offset,
                             [stv.ap[0], [W, ncl], [2, W // 2], [1, 2]])
                    io = iota16[:, :]
                    io4 = AP(io.tensor, io.offset,
                             [io.ap[0], [0, ncl], [2, W // 2], [1, 2]])
                    sc = scol_t[:, 2 * c0: 2 * (c0 + ncl)]
                    sc4 = AP(sc.tensor, sc.offset,
                             [sc.ap[0], [2, ncl], [0, W // 2], [1, 2]])
                    sv = sval_t[:, 2 * c0: 2 * (c0 + ncl)]
                    sv4 = AP(sv.tensor, sv.offset,
                             [sv.ap[0], [2, ncl], [0, W // 2], [1, 2]])
                    nc.vector.tensor_tensor(
                        out=st4, in0=io4, in1=sc4,
                        op=mybir.AluOpType.is_equal)
                    nc.vector.tensor_tensor(
                        out=st4, in0=st4, in1=sv4,
                        op=mybir.AluOpType.mult)
                    strips[W] = st

                y_ps = psy.tile([P, SGR], f32, space="PSUM", tag="y")
                nc.scalar.memzero(y_ps[:, :nrow])
                nmm = len(pm)
                for i, (w, b, b0, W, wi, ci, gc, p_lo, p_hi) in enumerate(pm):
                    gbuf = t01 if w < 2 else t23
                    gb = sg_bof[s][w] + b
                    nc.tensor.matmul(
                        y_ps[:, b0: b0 + W], lhsT=gbuf[:, gb, :],
                        rhs=strips[W][:, ci, :],
                        start=False, stop=(i == nmm - 1),
                        skip_group_check=True,
                    )
                yT = wpool.tile([P, SGR], f16, tag="yT")
                nc.scalar.copy(yT[:, :nrow], y_ps[:, :nrow])
                x_ps = psx.tile([P, SGR], f32, space="PSUM", tag="x")
                for t in range(ntl):
                    nc.tensor.matmul(x_ps[:, t * 128: (t + 1) * 128],
                                     lhsT=yT[:, t * 128: (t + 1) * 128],
                                     rhs=w_t, start=True, stop=True)
                xb = npool.tile([P, SG_TILES, P], f16, tag="xb")
                bt = b_t[:, :]
                bt4 = AP(bt.tensor, bt.offset, [bt.ap[0], [0, ntl], [1, P]])
                xp = x_ps[:, :nrow]
                xp3 = AP(xp.tensor, xp.offset,
                         [xp.ap[0], [P, ntl], [1, P]])
                nc.vector.tensor_tensor(
                    out=xb[:, :ntl, :], in0=xp3, in1=bt4,
                    op=mybir.AluOpType.add)
                r0 = s * SGR
                if xsh_fn is not None:
                    nc.sync.dma_start(
                        out=xsh_fn(s, ntl).rearrange(
                            "(c p) d -> p c d", p=P),
                        in_=xb[:, :ntl, :])
                norm_rows(xb, ntl, xn_out[hop, r0: r0 + ntl * 128, :])

            def graph_hop1(m, w_t, b_t, xsh_fn, hop, post_gather=None):
                for s in range(NSG):
                    n01 = o01[s + 1] - o01[s]
                    n23 = o23[s + 1] - o23[s]
                    t01 = g01pool.tile([P, T01, P], f16, tag="g01",
                                       name="t01")
                    t23 = g23pool.tile([P, T23, P], f16, tag="g23",
                                       name="t23")
                    if n01:
                        nc.sync.dma_start(
                            out=t01[:, :n01, :],
                            in_=x0g01[:, o01[s]: o01[s + 1], :])
                    if n23:
                        nc.sync.dma_start(
                            out=t23[:, :n23, :],
                            in_=x0g23[:, o23[s]: o23[s + 1], :])
                    if post_gather is not None:
                        post_gather(s)
                    compute_sg(m, s, t01, t23, w_t, b_t, xsh_fn, hop)

            def graph_hop2(m, srcs, w_t, b_t, hop):
                """hop 2: AGb fired first (CC waits internally); win-0/1
                gathers (xg1a, ready right after h1) prefetch 3 sgs deep;
                pair0 fills the AGb window; win-2/3 gathers stream with
                compute."""
                qload = [0, 0, 0, 0]
                t01s = {}
                t23s = {}
                npre = min(3, NSG)
                emit_ag(1)
                for s in range(npre):
                    t01s[s] = g01pool.tile([P, T01, P], f16, tag="g01",
                                           name="t01")
                    gather_chunks(m, srcs, s, t01s[s], None, [0, 1], qload)
                pair0_stage()
                for s in range(NSG):
                    t23s[s] = g23pool.tile([P, T23, P], f16, tag="g23",
                                           name="t23")
                    gather_chunks(m, srcs, s, None, t23s[s], [2, 3], qload)
                    compute_sg(m, s, t01s[s], t23s[s], w_t, b_t, None, hop)
                    if s + npre < NSG:
                        t01s[s + npre] = g01pool.tile(
                            [P, T01, P], f16, tag="g01", name="t01")
                        gather_chunks(m, srcs, s + npre, t01s[s + npre],
                                      None, [0, 1], qload)

            def xsh1_fn(s, ntl):
                if s < 6:
                    r = s * SGR
                    return xsh1a[r: r + ntl * 128, :]
                r = (s - 6) * SGR
                return xsh1b[r: r + ntl * 128, :]

            srcs2 = [xg1a[0: WIN, :], xg1a[WIN: NA, :],
                     xg1b[0: WIN, :], xg1b[WIN: NB, :]]

            def h1_hook(s):
                # trigger the A-half AllGather once its inputs exist
                if s == 6:
                    emit_ag(0)

            stages = os.environ.get(
                "BASS_GNN_STAGES", "p0,h1,h2").split(",")
            if "h1" in stages:
                graph_hop1(mA, w1_t, b1_t, xsh1_fn, 1,
                           post_gather=h1_hook)
            # overwrite metadata SBUF with hop-2 tables (Tile orders these
            # after hop-1's last reads)
            nc.sync.dma_start(out=scol_t[:, : 2 * mB["NPIECE"]],
                              in_=scolB[:, :])
            nc.sync.dma_start(out=sval_t[:, : 2 * mB["NPIECE"]],
                              in_=svalB[:, :])
            if "h2" in stages:
                graph_hop2(mB, srcs2, w2_t, b2_t, 2)
            elif "p0" in stages:
                pair0_stage()

    nc.compile()
    return nc


def _install_ntff_shim():
    """Provide antenv.axon_hooks (missing on this image) so trace=True can
    capture NTFF profiles through the axon .so."""
    import types
    if "antenv.axon_hooks" in sys.modules:
        return
    mod = types.ModuleType("antenv.axon_hooks")
    mod._hook = None

    def set_axon_ntff_profile_hook(h):
        mod._hook = h

    def get_axon_ntff_profile_hook():
        return mod._hook

    mod.set_axon_ntff_profile_hook = set_axon_ntff_profile_hook
    mod.get_axon_ntff_profile_hook = get_axon_ntff_profile_hook
    sys.modules["antenv.axon_hooks"] = mod
    try:
        from trn_agent_boot.trn_boot import _ntff_profile_via_ctypes
        mod._hook = _ntff_profile_via_ctypes("/opt/axon/libaxon_pjrt.so")
    except Exception:
        mod._hook = None


def _interleave(x):
    """[NPAD, D] global row order -> [A | B1 | B2] gather layout."""
    xs = x.reshape(NCORES, SHARD, -1)
    a = xs[:, :HA].reshape(NA, -1)
    b1 = xs[:, HA: HA + HB1].reshape(NB1, -1)
    b2 = xs[:, HA + HB1:].reshape(NB2, -1)
    return np.concatenate([a, b1, b2], axis=0)


def kernel(node_emb, attri_emb, W1, b1, W2, b2, edge_val,
           edge_row, edge_col, pos_src, pos_dst, neg_src, neg_dst):
    global LAST_RESULTS
    _install_ntff_shim()
    from concourse.bass_utils import run_bass_kernel_spmd

    pair_idx = np.concatenate([pos_src, pos_dst, neg_src, neg_dst]).astype(
        np.int64)
    structure, meta = _prep(edge_row.astype(np.int64),
                            edge_col.astype(np.int64), edge_val, pair_idx)

    import time as _time
    key = (structure, os.environ.get("BASS_GNN_STAGES", ""))
    if key in _CACHE:
        nc = _CACHE[key]
    else:
        t0 = _time.time()
        nc = _build_program(structure, meta)
        print(f"[kernel] build+schedule: {_time.time() - t0:.1f}s, "
              f"{len(nc.inst_map)} instructions", flush=True)
        _CACHE[key] = nc

    x0 = np.concatenate([node_emb, attri_emb], axis=0).astype(np.float32)
    x0p = np.zeros((NPAD, D), np.float32)
    x0p[:N] = x0
    x0f16 = x0p.astype(np.float16)

    mA, mB = meta["h1"], meta["h2"]
    in_maps = []
    for c in range(NCORES):
        gcols = mA["gcols_arrs"][c]
        rows = np.zeros((mA["NBLKT"] * 128, D), np.float16)
        sel = gcols >= 0
        rows[sel] = x0f16[gcols[sel]]
        blocks = rows.reshape(mA["NBLKT"], 128, D)
        b01 = []
        b23 = []
        goff = 0
        for s in range(NSG):
            n01 = int(mA["nblk01"][s])
            n23 = int(mA["nblk23"][s])
            b01.append(blocks[goff: goff + n01])
            b23.append(blocks[goff + n01: goff + n01 + n23])
            goff += n01 + n23
        x0g01 = np.ascontiguousarray(
            np.concatenate(b01, 0).transpose(1, 0, 2))
        x0g23 = np.ascontiguousarray(
            np.concatenate(b23, 0).transpose(1, 0, 2))
        in_maps.append({
            "x0sh": x0f16[c * SHARD: (c + 1) * SHARD],
            "x0g01": x0g01,
            "x0g23": x0g23,
            "scolA": mA["scol_arrs"][c],
            "svalA": mA["sval_arrs"][c],
            "gidxB": mB["gidx_arrs"][c],
            "scolB": mB["scol_arrs"][c],
            "svalB": mB["sval_arrs"][c],
            "w1": W1.astype(np.float16),
            "w2": W2.astype(np.float16),
            "brep1": np.broadcast_to(
                b1.astype(np.float32)[None, :], (P, D)).copy(),
            "brep2": np.broadcast_to(
                b2.astype(np.float32)[None, :], (P, D)).copy(),
        })

    trace = os.environ.get("BASS_GNN_TRACE", "0") == "1"
    t0 = _time.time()
    res = run_bass_kernel_spmd(nc, in_maps, core_ids=list(range(NCORES)),
                               trace=trace)
    print(f"[kernel] compile+run: {_time.time() - t0:.1f}s", flush=True)
    LAST_RESULTS = res

    # ---- host assembly: index normalized tables per hop ----
    xn_full = np.empty((3, NPAD, D), np.float32)
    for c in range(NCORES):
        xn_full[:, c * SHARD: (c + 1) * SHARD] = \
            res.results[c]["xn"].astype(np.float32)
    out = np.empty((4, 3, E_PAIR, D), np.float32)
    for st, idx in enumerate((pos_src, pos_dst, neg_src, neg_dst)):
        idx64 = idx.astype(np.int64)
        for h in range(3):
            out[st, h] = xn_full[h, idx64]
    return out
